# revision 17
# baseline (speedup 1.0000x reference)
"""DecoderAttentionGRU Trainium2 Bass/Tile kernel.

Per-core problem (batch shard Bs=16 of B=128, data-parallel over 8 cores):
  for t in 0..T-1 (T=256 sequential steps):
    y  = tanh(s@Wy1+by1); y = tanh(y@Wy2+by2); y = y@Wy3+by3          [Bs,O]
    sp = s@We1_s + be1                                                 [Bs,H]
    e  = squeeze(tanh(h_proj + sp[:,None,:]) @ We2) (+be2: dropped --
         softmax is shift-invariant)                                   [Bs,T]
    a  = softmax(e); c = einsum('bt,bth->bh', a, h)                    [Bs,H]
    GRU: x=[y,c]; r=sig(xWxr+sWhr+br); u=sig(xWxu+sWhu+bu)
         hcand=tanh(xWxh+(r*s)Whh); s=(1-u)hcand+u*s
  h_proj = h @ We1_h precomputed once (prologue).

Layouts are feature-on-partition ("T-layout"): activations stored as
[128, (chunk, batch)] so matmuls are option-B (weights stationary,
batch streaming) and gate nonlinearities use all 128 ACT lanes.

sigmoid avoided (ACT table-set switch): sig(v) = 0.5*(1+tanh(v/2));
r*s folded as  (r*s)@Whh = s@(Whh/2) + (tanh(pre_r/2)*s)@(Whh/2);
u-gate: s_new = hc + 0.5*(d + tanh(pre_u/2)*d), d = s - hc.
"""

import numpy as np
from contextlib import ExitStack

import concourse.bass as bass
import concourse.tile as tile
from concourse import bacc, mybir
from concourse.bass import ds, ts
from concourse.masks import make_identity

F16 = mybir.dt.float16
F32 = mybir.dt.float32
F8 = mybir.dt.float8e4
AF = mybir.ActivationFunctionType
ALU = mybir.AluOpType
AX = mybir.AxisListType

P = 128
B, T, H, O = 128, 256, 512, 256
X = O + H            # 768
NCORES = 8
Bs = B // NCORES     # 16
HC, TC, OC, XC = H // P, T // P, O // P, X // P   # 4, 2, 2, 6


def build_nc(nsteps=T, kf=8, dynamic=True, bias_on=None, dump=False, skip=frozenset(),
             reps=1):
    """Build and compile the per-core Bass module.

    bias_on: dict name->bool; biases that are identically zero in the input
    are skipped (value-independent instruction stream otherwise).
    """
    if bias_on is None:
        bias_on = {k: True for k in ("by1", "by2", "by3", "be1", "br", "bu")}
    assert nsteps % kf == 0

    nc = bacc.Bacc("TRN2", target_bir_lowering=False, debug=False)

    h_d = nc.dram_tensor("h", [Bs, T, H], F32, kind="ExternalInput")
    s0_d = nc.dram_tensor("s0", [Bs, H], F32, kind="ExternalInput")
    wd = {}
    for name, shape in [
        ("Wy1", [H, H]), ("by1", [H]), ("Wy2", [H, H]), ("by2", [H]),
        ("Wy3", [H, O]), ("by3", [O]), ("We1", [2 * H, H]), ("be1", [H]),
        ("We2", [H, 1]), ("be2", [1]),
        ("Wxr", [X, H]), ("Whr", [H, H]), ("br", [H]),
        ("Wxu", [X, H]), ("Whu", [H, H]), ("bu", [H]),
        ("Wxh", [X, H]), ("Whh", [H, H]),
    ]:
        wd[name] = nc.dram_tensor(name, shape, F32, kind="ExternalInput")
    out_d = nc.dram_tensor("out", [Bs, T, O], F32, kind="ExternalOutput")
    dbg = {}
    if dump:
        for nm, shape in [("d_asb", [P, 2, T]), ("d_aT16", [P, TC * Bs]),
                          ("d_xT", [P, XC * Bs]), ("d_psb", [P, 2, T]),
                          ("d_spf", [P, HC * Bs]), ("d_eps", [P, 4, T]),
                          ("d_esum", [P, 4]), ("d_rcp", [P, 4])]:
            dbg[nm] = nc.dram_tensor(nm, shape, F32, kind="ExternalOutput")

    with tile.TileContext(nc) as tc, ExitStack() as top:
        # ---------------- persistent SBUF ----------------
        pconst = top.enter_context(tc.tile_pool(name="const", bufs=1))
        pstate = top.enter_context(tc.tile_pool(name="state", bufs=1))

        ident16 = pconst.tile([P, P], F16, tag="id16", name="id16")
        identf32 = pconst.tile([P, P], F32, tag="id32", name="id32")
        make_identity(nc, ident16)
        make_identity(nc, identf32)
        ones16 = pconst.tile([1, Bs], F16, tag="ones", name="ones")
        nc.vector.memset(ones16, 1.0)

        # fp16 weights, T-layout [p, kc, m]
        w16 = {}
        for name, kc, m in [("Wy1", HC, H), ("Wy2", HC, H), ("Wy3", HC, O),
                            ("Wxr", XC, H), ("Wxu", XC, H), ("Wxh", XC, H),
                            ("Whr", HC, H), ("Whu", HC, H), ("Whh", HC, H)]:
            w16[name] = pconst.tile([P, kc, m], F16, tag=f"w_{name}", name=f"w_{name}")
        w16["We1s"] = pconst.tile([P, HC, H], F16, tag="w_We1s", name="w_We1s")
        we2_16 = pconst.tile([P, HC], F16, tag="w_We2", name="w_We2")
        b16 = {}
        for name, m in [("by1", H), ("by2", H), ("by3", O), ("be1", H),
                        ("br", H), ("bu", H)]:
            if bias_on[name]:
                b16[name] = pconst.tile([1, m], F16, tag=f"b_{name}", name=f"b_{name}")

        h16 = pstate.tile([P, Bs, TC, H], F16, tag="h16", name="h16")       # [t%128,(b,tc,h)]
        h8 = pstate.tile([P, Bs, TC, H], F8, tag="h8", name="h8")          # fp8 copy for cdot
        hp = pstate.tile([P, HC, Bs, T], F16, tag="hp", name="hp")         # [h'%128,(c,b,t)]
        s_f32 = pstate.tile([P, HC * Bs], F32, tag="s", name="s")         # [h%128, c*16+b]

        # ---------------- prologue ----------------
        # phase 1: weights (staging pool scoped, released before phase 2)
        with tc.tile_pool(name="stage", bufs=2) as pstg:

            def load_w16(dst, dram_ap, kc, m, scale=None):
                stg = pstg.tile([P, kc, m], F32, tag="wstage", name="wstage")
                nc.sync.dma_start(
                    stg[:, :kc, :m],
                    dram_ap.rearrange("(kc p) m -> p kc m", p=P))
                src = stg[:, :kc, :m].rearrange("p kc m -> p (kc m)")
                dstv = dst.rearrange("p kc m -> p (kc m)")
                if scale is None:
                    nc.vector.tensor_copy(dstv, src)
                else:
                    nc.vector.tensor_scalar_mul(dstv, src, scale)

            for name in ("Wy1", "Wy2", "Wxr", "Wxu", "Wxh", "Whr", "Whu"):
                d = wd[name]
                kc = d.shape[0] // P
                load_w16(w16[name], d[:, :], kc, d.shape[1])
            load_w16(w16["Whh"], wd["Whh"][:, :], HC, H, scale=0.5)
            load_w16(w16["Wy3"], wd["Wy3"][:, :], HC, O)
            load_w16(w16["We1s"], wd["We1"][H:, :], HC, H)
            # We2 [H,1] -> [p, c]
            stg = pstg.tile([P, HC], F32, tag="we2stage", name="we2stage")
            nc.sync.dma_start(stg[:], wd["We2"][:, 0].rearrange(
                "(c p) -> p c", p=P))
            nc.vector.tensor_copy(we2_16[:], stg[:])
            for name in b16:
                m = b16[name].shape[1]
                stg = pstg.tile([1, m], F32, tag="bstage", name="bstage")
                nc.sync.dma_start(stg[:1, :m], wd[name][None, :])
                nc.vector.tensor_copy(b16[name][:], stg[:1, :m])

        # phase 2: h, s0, h_proj
        with tc.tile_pool(name="stage2", bufs=2) as pstg, \
             tc.tile_pool(name="pps", bufs=2, space="PSUM") as pps, \
             tc.tile_pool(name="hTb", bufs=2) as phT:

            # h: [Bs,T,H] f32 -> h16 [p,(b,tc,h)] f16, in 4-batch chunks
            for bb in range(0, Bs, 4):
                h32 = pstg.tile([P, 4, TC, H], F32, tag="h32", name="h32")
                nc.sync.dma_start(
                    h32[:], h_d[bb:bb + 4, :, :].rearrange(
                        "b (tc p) hh -> p b tc hh", p=P))
                nc.vector.tensor_copy(
                    h16[:, bb:bb + 4].rearrange("p b tc hh -> p (b tc hh)"),
                    h32.rearrange("p b tc hh -> p (b tc hh)"))
                nc.vector.tensor_copy(
                    h8[:, bb:bb + 4].rearrange("p b tc hh -> p (b tc hh)"),
                    h32.rearrange("p b tc hh -> p (b tc hh)"))

            # s0 -> s_f32 (transpose via PE)
            s0stg = pstg.tile([Bs, H], F32, tag="s0stage", name="s0stage")
            nc.sync.dma_start(s0stg[:], s0_d[:, :])
            for c in range(HC):
                ps = pps.tile([P, Bs], F32, tag="s0ps", name="s0ps")
                nc.tensor.transpose(ps[:], s0stg[:, c * P:(c + 1) * P],
                                    identf32[:Bs, :Bs])
                nc.vector.tensor_copy(s_f32[:, c * Bs:(c + 1) * Bs], ps[:])

            # We1_h fp16
            w1h = pstg.tile([P, HC, H], F16, tag="w1hstage", name="w1hstage")
            stg = pstg.tile([P, HC, H], F32, tag="w1hstg32", name="w1hstg32")
            nc.sync.dma_start(stg[:], wd["We1"][:H, :].rearrange(
                "(kc p) m -> p kc m", p=P))
            nc.vector.tensor_copy(
                w1h.rearrange("p kc m -> p (kc m)"),
                stg.rearrange("p kc m -> p (kc m)"))

            # hT block (4 batches) -> h_proj block; hp[p,(m,b,t)]
            for bb in range(0, Bs, 4):
                hT = phT.tile([P, HC, 4, T], F16, tag="hT", name="hT")
                for b4 in range(4):
                    for tcc in range(TC):
                        for c in range(HC):
                            ps = pps.tile([P, P], F16, tag="hTps", name="hTps")
                            nc.tensor.transpose(
                                ps[:], h16[:, bb + b4, tcc, c * P:(c + 1) * P],
                                ident16)
                            nc.vector.tensor_copy(
                                hT[:, c, b4, tcc * P:(tcc + 1) * P], ps[:])
                hTv = hT.rearrange("p c b t -> p c (b t)")
                for m in range(HC):
                    for nb in range(2):   # 4*T/512
                        ps = pps.tile([P, 512], F32, tag="hpps", name="hpps")
                        for k in range(HC):
                            nc.tensor.matmul(
                                ps[:], w1h[:, k, m * P:(m + 1) * P],
                                hTv[:, k, nb * 512:(nb + 1) * 512],
                                start=(k == 0), stop=(k == HC - 1))
                        nc.vector.tensor_copy(
                            hp[:, m, bb:bb + 4].rearrange("p b t -> p (b t)")[
                                :, nb * 512:(nb + 1) * 512], ps[:])

        # ---------------- steady-state pools ----------------
        pz = top.enter_context(tc.tile_pool(name="z", bufs=3))
        psm = top.enter_context(tc.tile_pool(name="small", bufs=2))
        pyb = top.enter_context(tc.tile_pool(name="ybuf", bufs=2))
        ppe = top.enter_context(tc.tile_pool(name="pe", bufs=1, space="PSUM"))
        ppy = top.enter_context(tc.tile_pool(name="py", bufs=2, space="PSUM"))
        ppg = top.enter_context(tc.tile_pool(name="pg", bufs=2, space="PSUM"))
        ppx = top.enter_context(tc.tile_pool(name="px", bufs=2, space="PSUM"))

        out_flat = out_d[:, :, :].rearrange("b t o -> b (t o)")

        # layout (r2, g, t): within a half g the two active accumulation
        # groups (r2=0,1) sit in different PSUM banks -- start=True clears
        # has_written for the written rows across the WHOLE bank, so two
        # groups interleaving their accumulation must not share a bank.
        e_ps = ppe.tile([P, 2, 2, T], F32, tag="eps", name="eps")
        nc.vector.memset(e_ps.rearrange("p a b t -> p (a b t)"), 0.0)

        def mm_block(out_ps, mcount, terms, bias=None, open_=True, close=True):
            """out_ps[:, m*16:(m+1)*16] += sum_terms W[k,m].T @ rhs[k] (+bias)

            open_: True  -> each m-group starts with start=True (safe only when
                            groups close before the next one opens: start=True
                            clears has_written for the written rows across the
                            WHOLE bank, killing any open sibling group);
                   "first" -> single start=True on the very first MM (one bank
                            clear, everything after accumulates; lets all
                            m-slices of this tile stay open concurrently);
                   False -> pure accumulation (a prior call opened the tile).
            close: whether the last MM of each m carries stop=True."""
            for m in range(mcount):
                seq = []
                for (w, kc, rhs) in terms:
                    for k in range(kc):
                        seq.append((w[:, k, m * P:(m + 1) * P],
                                    rhs[:, k * Bs:(k + 1) * Bs]))
                if bias is not None:
                    seq.append((bias[:1, m * P:(m + 1) * P], ones16[:1, :]))
                n = len(seq)
                for i, (lhsT, rhs) in enumerate(seq):
                    st = (open_ is True and i == 0) or \
                         (open_ == "first" and m == 0 and i == 0)
                    nc.tensor.matmul(out_ps[:, m * Bs:(m + 1) * Bs], lhsT, rhs,
                                     start=st,
                                     stop=(close and i == n - 1))

        def dump16(dst, src_ap):
            tmp = psm.tile(list(src_ap.shape), F32, tag="dmp", name="dmp")
            nc.vector.tensor_copy(tmp[:], src_ap)
            nc.sync.dma_start(dst[tuple(slice(None) for _ in src_ap.shape)], tmp[:])

        def emit_step(ksub, y_buf, first=False):
            # --- head: y = dense3(s); sp = s@We1s + be1 ---
            sT16 = psm.tile([P, HC * Bs], F16, tag="sT16", name="sT16")
            nc.vector.tensor_copy(sT16[:], s_f32[:])

            sp_ps = ppx.tile([P, HC * Bs], F32, tag="xps", name="xps")
            mm_block(sp_ps, HC, [(w16["We1s"], HC, sT16)], b16.get("be1"))
            sp_f = psm.tile([P, HC * Bs], F32, tag="sp_f", name="sp_f")
            nc.vector.tensor_copy(sp_f[:], sp_ps[:])

            y1ps = ppy.tile([P, HC * Bs], F32, tag="yps", name="yps")
            mm_block(y1ps, HC, [(w16["Wy1"], HC, sT16)], b16.get("by1"))
            y1 = psm.tile([P, HC * Bs], F16, tag="y1", name="y1")
            nc.scalar.activation(y1[:], y1ps[:], AF.Tanh)
            y2ps = ppy.tile([P, HC * Bs], F32, tag="yps", name="yps")
            mm_block(y2ps, HC, [(w16["Wy2"], HC, y1)], b16.get("by2"))
            y2 = psm.tile([P, HC * Bs], F16, tag="y2", name="y2")
            nc.scalar.activation(y2[:], y2ps[:], AF.Tanh)
            y3ps = ppy.tile([P, OC * Bs], F32, tag="yps", name="yps")
            mm_block(y3ps, OC, [(w16["Wy3"], HC, y2)], b16.get("by3"))
            yT3 = psm.tile([P, OC * Bs], F32, tag="yT3", name="yT3")
            nc.vector.tensor_copy(yT3[:], y3ps[:])
            xT = psm.tile([P, XC * Bs], F16, tag="xT", name="xT")
            nc.vector.tensor_copy(xT[:, : OC * Bs], y3ps[:])

            # r-gate s-side terms pre-run: emitted before the attention
            # section so the in-order PE accumulates them while ACT owns the
            # critical path (z tanh).  Single start (open_="first") keeps all
            # m-slices of rps open concurrently; x-side terms close it later.
            # ups is NOT pre-run: its start would clear rps' has_written if
            # the pool packs both bufs into one PSUM bank.
            rps = ppg.tile([P, HC * Bs], F32, tag="gps", name="gps")
            gru_skip = "gru" in skip
            mm_block(rps, HC, [(w16["Whr"], HC, sT16)], b16.get("br"),
                     open_="first", close=gru_skip)

            # --- attention ---
            # e-dot is 4x col-tiled: col-group j = b//4 -> psum row 32j,
            # free slice (r = b%4, t).  Softmax runs on all 128 psum rows
            # (124 garbage rows compute in parallel lanes for free; e_ps is
            # memset once so garbage stays finite).  aT = gather+transpose
            # in one matmul: stationary a_full block, streaming 4 identity
            # columns ident16[:, 0:128:32] selects rows {32j}.
            # aT16 column = tc*16 + rho(b), rho(b) = (b%4)*4 + b//4.
            p_sb = psm.tile([P, 2, T], F32, tag="psb", name="psb")
            a_sb = psm.tile([P, 2, T], F16, tag="asb", name="asb")
            esum = psm.tile([P, 4], F32, tag="esum", name="esum")
            rcp = psm.tile([P, 4], F32, tag="rcp", name="rcp")
            aT_ps = ppx.tile([P, TC * Bs], F32, tag="xps", name="xps")
            aT16 = psm.tile([P, TC * Bs], F8, tag="aT16", name="aT16")
            cT_ps = ppy.tile([P, HC * Bs], F32, tag="yps", name="yps")

            for g in range(2):            # halves: r = b%4 in {2g, 2g+1}
                half = [4 * j + 2 * r2 + g for r2 in range(2) for j in range(4)]
                for c in range(HC):
                    zin = pz.tile([P, 8 * T], F16, tag="zin", name="zin")
                    if "z" not in skip:
                        for sl, b in enumerate(half):
                            nc.vector.tensor_scalar_add(
                                zin[:, sl * T:(sl + 1) * T], hp[:, c, b, :],
                                sp_f[:, c * Bs + b: c * Bs + b + 1])
                    z = pz.tile([P, 8 * T], F16, tag="z", name="z")
                    if "z" not in skip:
                        nc.scalar.activation(z[:], zin[:], AF.Tanh)
                    else:
                        nc.vector.memset(z[:], 0.0)
                    if "edot" in skip:
                        continue
                    for sl, b in enumerate(half):
                        j, r = divmod(b, 4)
                        nc.tensor.matmul(
                            e_ps[32 * j: 32 * j + 1, r // 2, g, :],
                            we2_16[:, c: c + 1],
                            z[:, sl * T:(sl + 1) * T],
                            start=(c == 0), stop=(c == HC - 1),
                            tile_position=(0, 32 * j))
                # softmax over t on every psum row (only rows 32j matter)
                if "edot" in skip:
                    nc.vector.memset(a_sb.rearrange("p a t -> p (a t)"), 0.004)
                else:
                    nc.scalar.activation(p_sb[:], e_ps[:, :, g, :], AF.Exp)
                if "edot" not in skip:
                    nc.vector.tensor_reduce(esum[:, g:4:2], p_sb[:],
                                            AX.X, ALU.add)
                if "edot" not in skip:
                    nc.vector.reciprocal(rcp[:, g:4:2], esum[:, g:4:2])
                for r2 in (() if "edot" in skip else range(2)):
                    rr4 = (2 * r2 + g) * 4
                    # a_sb holds 64*a: keeps the fp8 aT16 copy inside e4m3's
                    # normal range (raw a ~ 1/T = 0.004 would be subnormal);
                    # the 1/64 is folded into the xT context copy below.
                    nc.vector.tensor_scalar(
                        a_sb[:, r2, :], p_sb[:, r2, :],
                        rcp[:, 2 * r2 + g: 2 * r2 + g + 1], 64.0,
                        ALU.mult, ALU.mult)
                    for tcc in range(TC):
                        nc.tensor.matmul(
                            aT_ps[:, tcc * Bs + rr4: tcc * Bs + rr4 + 4],
                            a_sb[:, r2, tcc * P:(tcc + 1) * P],
                            ident16[:, 0:P:32],
                            start=True, stop=True)
                if "edot" not in skip:
                    nc.vector.tensor_copy(
                        aT16.rearrange("p (tc q four) -> p tc q four", tc=TC, q=4)[
                            :, :, g::2, :],
                        aT_ps.rearrange("p (tc q four) -> p tc q four", tc=TC, q=4)[
                            :, :, g::2, :])
                else:
                    nc.vector.memset(
                        aT16.rearrange("p tcb -> p (tcb)"), 0.004)
                # context c^T for this half
                for b in (() if "cdot" in skip else half):
                    rr = (b % 4) * 4 + b // 4
                    for c2 in range(HC):
                        for tcc in range(TC):
                            nc.tensor.matmul(
                                cT_ps[:, c2 * Bs + b: c2 * Bs + b + 1],
                                h8[:, b, tcc, c2 * P:(c2 + 1) * P],
                                aT16[:, tcc * Bs + rr: tcc * Bs + rr + 1],
                                start=(tcc == 0), stop=(tcc == TC - 1))
            if dump and first:
                dump16(dbg["d_asb"], a_sb[:])
                dump16(dbg["d_aT16"], aT16[:])
                dump16(dbg["d_psb"], p_sb[:])
                dump16(dbg["d_spf"], sp_f[:])
                dump16(dbg["d_esum"], esum[:])
                dump16(dbg["d_rcp"], rcp[:])
                dump16(dbg["d_eps"], e_ps[:])
            if "cdot" in skip:
                nc.vector.memset(xT[:, OC * Bs:], 0.004)
            else:
                nc.vector.tensor_scalar_mul(xT[:, OC * Bs:], cT_ps[:], 1.0 / 64.0)
            if dump and first:
                dump16(dbg["d_xT"], xT[:])

            # --- GRU --- (r x-side terms close the pre-run tile)
            ups = ppg.tile([P, HC * Bs], F32, tag="gps", name="gps")
            if not gru_skip:
                mm_block(rps, HC, [(w16["Wxr"], XC, xT)], open_=False)
                mm_block(ups, HC, [(w16["Wxu"], XC, xT), (w16["Whu"], HC, sT16)],
                         b16.get("bu"))
            else:
                mm_block(ups, HC, [(w16["Whu"], HC, sT16)])
            tr = psm.tile([P, HC * Bs], F32, tag="tr", name="tr")
            nc.scalar.activation(tr[:], rps[:], AF.Tanh, scale=0.5)
            trs = psm.tile([P, HC * Bs], F16, tag="trs", name="trs")
            nc.vector.tensor_tensor(trs[:], tr[:], s_f32[:], ALU.mult)
            hps = ppg.tile([P, HC * Bs], F32, tag="gps", name="gps")
            if "gru" not in skip:
                mm_block(hps, HC, [(w16["Wxh"], XC, xT), (w16["Whh"], HC, sT16),
                                   (w16["Whh"], HC, trs)])
            else:
                mm_block(hps, HC, [(w16["Whh"], HC, trs)])
            tu = psm.tile([P, HC * Bs], F32, tag="tu", name="tu")
            nc.scalar.activation(tu[:], ups[:], AF.Tanh, scale=0.5)
            hcand = psm.tile([P, HC * Bs], F32, tag="hcand", name="hcand")
            nc.scalar.activation(hcand[:], hps[:], AF.Tanh)
            # s_new = hcand + 0.5*(1+tanh(pre_u/2))*(s - hcand), fused to 3 ops
            d_ = psm.tile([P, HC * Bs], F32, tag="d_", name="d_")
            nc.vector.tensor_tensor(d_[:], s_f32[:], hcand[:], ALU.subtract)
            q_ = psm.tile([P, HC * Bs], F32, tag="q_", name="q_")
            nc.vector.scalar_tensor_tensor(q_[:], tu[:], 1.0, d_[:],
                                           ALU.add, ALU.mult)
            nc.vector.scalar_tensor_tensor(s_f32[:], q_[:], 0.5, hcand[:],
                                           ALU.mult, ALU.add)

            # --- output y (fp32) ---
            yT_ps = ppx.tile([Bs, O], F32, tag="xps", name="xps")
            for oc in range(OC):
                nc.tensor.transpose(yT_ps[:, oc * P:(oc + 1) * P],
                                    yT3[:, oc * Bs:(oc + 1) * Bs], identf32)
            nc.vector.tensor_copy(y_buf[:, ksub, :], yT_ps[:])

        if dynamic:
            def body(iv0, unroll):
                assert unroll == kf
                y_buf = pyb.tile([Bs, kf, O], F32, tag="ybuf", name="ybuf")
                for k in range(kf):
                    emit_step(k, y_buf)
                nc.sync.dma_start(
                    out_flat[:, ds(iv0 * O, kf * O)],
                    y_buf.rearrange("b k o -> b (k o)"))
            for _rep in range(reps):
                tc.For_i_unrolled_general(
                    0, nsteps, 1, body, max_unroll=kf,
                    hint_engines=(mybir.EngineType.PE, mybir.EngineType.Activation,
                                  mybir.EngineType.DVE, mybir.EngineType.SP,
                                  mybir.EngineType.Pool))
        else:
            for t0 in range(0, nsteps, kf):
                y_buf = pyb.tile([Bs, kf, O], F32, tag="ybuf", name="ybuf")
                for k in range(kf):
                    emit_step(k, y_buf, first=(t0 == 0 and k == 0))
                nc.sync.dma_start(
                    out_flat[:, t0 * O: (t0 + kf) * O],
                    y_buf.rearrange("b k o -> b (k o)"))

    nc.compile()
    return nc


def shard_inputs(inputs, nsteps=T):
    """Full inputs -> per-core in_maps (batch-sharded h/s0, replicated W)."""
    maps = []
    for c in range(NCORES):
        m = {}
        sl = slice(c * Bs, (c + 1) * Bs)
        for k, v in inputs.items():
            v = np.asarray(v, dtype=np.float32)
            m[k] = v[sl] if k in ("h", "s0") else v
        maps.append(m)
    return maps


def bias_flags(inputs):
    return {k: bool(np.any(np.asarray(inputs[k]) != 0))
            for k in ("by1", "by2", "by3", "be1", "br", "bu")}


# ----------------------------------------------------------------------
# harness entry point: full inputs in, full output out
# ----------------------------------------------------------------------
_CACHE = {}


def _get_nc(bias_key):
    if bias_key not in _CACHE:
        _CACHE[bias_key] = build_nc(
            nsteps=T, kf=8, dynamic=True,
            bias_on=dict(zip(("by1", "by2", "by3", "be1", "br", "bu"),
                             bias_key)))
    return _CACHE[bias_key]


def kernel(**inputs) -> np.ndarray:
    """Full [B,T,H]-style inputs -> full [B,T,O] float32 output."""
    from concourse.bass_utils import run_bass_kernel_spmd

    flags = bias_flags(inputs)
    nc = _get_nc(tuple(flags[k] for k in ("by1", "by2", "by3", "be1", "br", "bu")))
    in_maps = shard_inputs(inputs)
    res = run_bass_kernel_spmd(nc, in_maps, list(range(NCORES)))
    out = np.concatenate([res.results[c]["out"] for c in range(NCORES)], axis=0)
    return out.astype(np.float32)

